# revision 3
# baseline (speedup 1.0000x reference)
"""CausalUnitRunEncoder — Trainium2 Bass kernel (8 NeuronCores, batch-parallel).

Self-contained: builds + compiles an SPMD Bass program on first call, shards
batch 32 -> 4 rows/core, runs on cores 0-7 via run_bass_kernel_spmd, gathers.

Layout: channel-major packed — SBUF tiles [128, C] hold 2 batch rows
(partitions 0:64 = one batch row's 64 channels, 64:128 = the next).
Per core: 2 batch-pairs x T=16384 tokens, in C-token chunks with a 16-token
causal halo carried between chunks.

Per chunk:
  - GPSIMD ap_gather: fused embedding+input-projection lookup from a
    host-precomputed table E2 = emb @ in_w[:, :64].T  (SBUF-resident, [128, 2048])
  - PE: rank-6 feature matmul + identity-add of the gather + mask broadcast
  - DVE scalar_tensor_tensor: h0 = (psum + in_b) * mask_bcast
  - 4 dilated conv layers: 3 diagonal-matmul taps (PE, PSUM-accumulated)
    -> Gelu with per-channel bias (ACT) -> blockdiag pointwise matmul (PE)
    -> residual via scalar_tensor_tensor (DVE)
  - LayerNorm: PE transpose to token-major, DVE reduces for sum/sumsq,
    small-tile stat chain, PE broadcast matmuls for the per-token scale
    terms, 3 DVE apply ops, PE transpose back, ACT evict, DMA out.
"""
import sys, os
sys.path.insert(0, '/opt/trn_rl_repo')
os.environ.setdefault('JAX_PLATFORMS', 'cpu')

from contextlib import ExitStack
import numpy as np

import concourse.bass as bass
import concourse.tile as tile
from concourse import bacc, mybir

F32 = mybir.dt.float32
I16 = mybir.dt.int16
AF = mybir.ActivationFunctionType
OP = mybir.AluOpType
AX = mybir.AxisListType

DIM = 64
VOCAB = 2048
K = 3
DILATIONS = (1, 2, 4, 8)
HALO = 16  # max lookback = dil*(K-1) = 16
P = 128
MM_N = 512  # max fp32 moving-operand columns per matmul
N_CORES = 8
T_FULL = 16384
C_CHUNK = 512


def build_program(T: int, C: int) -> bass.Bass:
    """SPMD program for one core: 2 batch-pairs x T tokens, chunks of C."""
    assert T % C == 0 and C % MM_N == 0 and C % 128 == 0
    NCH = T // C          # chunks per pair
    NB = C // 128         # 128-token blocks per chunk
    NS = C // 16          # idx cols per chunk

    nc = bacc.Bacc("TRN2", target_bir_lowering=False, debug=False)

    d_anchor = nc.declare_dram_parameter("anchor", [4, T], F32, isOutput=False)
    d_rate = nc.declare_dram_parameter("rate", [4, T], F32, isOutput=False)
    d_sil = nc.declare_dram_parameter("sil", [4, T], F32, isOutput=False)
    d_mask = nc.declare_dram_parameter("mask", [4, T], F32, isOutput=False)
    d_idxw = nc.declare_dram_parameter("idxw", [2, NCH, P, NS], I16, isOutput=False)
    d_etab = nc.declare_dram_parameter("etab", [P, VOCAB], F32, isOutput=False)
    d_w6 = nc.declare_dram_parameter("w6", [6, P], F32, isOutput=False)
    d_sel2 = nc.declare_dram_parameter("sel2", [2, P], F32, isOutput=False)
    d_inb2 = nc.declare_dram_parameter("inb2", [P, 1], F32, isOutput=False)
    d_dtaps = nc.declare_dram_parameter("dtaps", [P, 12 * P], F32, isOutput=False)
    d_pw2 = nc.declare_dram_parameter("pw2", [P, 4 * P], F32, isOutput=False)
    d_dwb2 = nc.declare_dram_parameter("dwb2", [P, 4], F32, isOutput=False)
    d_pwb2 = nc.declare_dram_parameter("pwb2", [P, 4], F32, isOutput=False)
    d_ident = nc.declare_dram_parameter("ident", [P, P], F32, isOutput=False)
    d_i2 = nc.declare_dram_parameter("i2", [2, 2], F32, isOutput=False)
    d_gsel = nc.declare_dram_parameter("gsel", [2, P], F32, isOutput=False)
    d_bsel = nc.declare_dram_parameter("bsel", [2, P], F32, isOutput=False)
    d_g2col = nc.declare_dram_parameter("g2col", [P, 1], F32, isOutput=False)
    d_out = nc.declare_dram_parameter("out", [4, T, DIM], F32, isOutput=True)

    with tile.TileContext(nc) as tc, ExitStack() as ctx:
        cpool = ctx.enter_context(tc.tile_pool(name="const", bufs=1))

        def cload(shape, dram, tag, dt=F32):
            t = cpool.tile(shape, dt, tag=tag)
            nc.sync.dma_start(t[:], dram[:])
            return t

        etab = cload([P, VOCAB], d_etab, "etab")
        w6 = cload([6, P], d_w6, "w6")
        sel2 = cload([2, P], d_sel2, "sel2")
        inb2 = cload([P, 1], d_inb2, "inb2")
        dtaps = cload([P, 12 * P], d_dtaps, "dtaps")
        pw2 = cload([P, 4 * P], d_pw2, "pw2")
        dwb2 = cload([P, 4], d_dwb2, "dwb2")
        pwb2 = cload([P, 4], d_pwb2, "pwb2")
        ident = cload([P, P], d_ident, "ident")
        i2 = cload([2, 2], d_i2, "i2")
        gsel = cload([2, P], d_gsel, "gsel")
        bsel = cload([2, P], d_bsel, "bsel")
        g2col = cload([P, 1], d_g2col, "g2col")
        epsc = cpool.tile([P, 1], F32)
        nc.vector.memset(epsc[:], 1e-5)

        def dtap(i, k):
            j = i * 3 + k
            return dtaps[:, j * P:(j + 1) * P]

        def pwt(i):
            return pw2[:, i * P:(i + 1) * P]

        rpool = ctx.enter_context(tc.tile_pool(name="rows", bufs=3))
        ipool = ctx.enter_context(tc.tile_pool(name="idx", bufs=3))
        egpool = ctx.enter_context(tc.tile_pool(name="eg", bufs=2))
        hpools = [ctx.enter_context(tc.tile_pool(name=f"h{i}", bufs=2)) for i in range(5)]
        gpool = ctx.enter_context(tc.tile_pool(name="g", bufs=2))
        spool = ctx.enter_context(tc.tile_pool(name="stats", bufs=2))
        opool = ctx.enter_context(tc.tile_pool(name="o", bufs=2))
        outpool = ctx.enter_context(tc.tile_pool(name="outt", bufs=3))

        psA = ctx.enter_context(tc.tile_pool(name="psA", bufs=2, space="PSUM"))
        psB = ctx.enter_context(tc.tile_pool(name="psB", bufs=2, space="PSUM"))
        psO = ctx.enter_context(tc.tile_pool(name="psO", bufs=2, space="PSUM"))
        psS = ctx.enter_context(tc.tile_pool(name="psS", bufs=1, space="PSUM"))

        def mm(psum_t_, lhsT, rhs, start, stop, col0=0):
            n = rhs.shape[-1]
            for off in range(0, n, MM_N):
                w = min(MM_N, n - off)
                nc.tensor.matmul(
                    psum_t_[:, col0 + off: col0 + off + w],
                    lhsT, rhs[:, off:off + w], start=start, stop=stop)

        for pair in range(2):
            b0, b1 = 2 * pair, 2 * pair + 1
            prev_h = [None] * 5
            for j in range(NCH):
                t0 = j * C
                # ---- input DMAs ----
                R = rpool.tile([6, C], F32, tag="rows")
                for row, dram in ((0, d_anchor), (1, d_rate), (2, d_sil)):
                    nc.sync.dma_start(R[row:row + 1, :], dram[b0:b0 + 1, t0:t0 + C])
                    nc.sync.dma_start(R[row + 3:row + 4, :], dram[b1:b1 + 1, t0:t0 + C])
                M2 = rpool.tile([2, C], F32, tag="m2")
                nc.sync.dma_start(M2[0:1, :], d_mask[b0:b0 + 1, t0:t0 + C])
                nc.sync.dma_start(M2[1:2, :], d_mask[b1:b1 + 1, t0:t0 + C])
                idxt = ipool.tile([P, NS], I16, tag="idx")
                nc.sync.dma_start(idxt[:], d_idxw[pair, j])

                # ---- gather (fused embed + input-proj lookup) ----
                eg = egpool.tile([P, C], F32, tag="eg")
                nc.gpsimd.ap_gather(eg[:], etab[:], idxt[:], channels=P,
                                    num_elems=VOCAB, d=1, num_idxs=C)

                # ---- input stage ----
                p_h = psA.tile([P, C], F32, tag="a")
                mm(p_h, w6[:], R[:], start=True, stop=False)
                mm(p_h, ident[:], eg[:], start=False, stop=True)
                p_m = psB.tile([P, C], F32, tag="b")
                mm(p_m, sel2[:], M2[:], start=True, stop=True)

                e1 = egpool.tile([P, C], F32, tag="e1")
                nc.scalar.activation(e1[:], p_h[:], AF.Identity, bias=inb2[:])
                h0 = hpools[0].tile([P, HALO + C], F32, tag="h0")
                nc.vector.tensor_mul(h0[:, HALO:], e1[:], p_m[:])
                if j == 0:
                    nc.vector.memset(h0[:, 0:HALO], 0.0)
                else:
                    nc.vector.tensor_copy(h0[:, 0:HALO], prev_h[0][:, C:C + HALO])

                # ---- conv layers ----
                h_in = h0
                for li, dil in enumerate(DILATIONS):
                    p_u = psA.tile([P, C], F32, tag="a")
                    for k in range(3):
                        sh = HALO - (2 - k) * dil
                        mm(p_u, dtap(li, k), h_in[:, sh:sh + C],
                           start=(k == 0), stop=(k == 2))
                    g = gpool.tile([P, C], F32, tag="g")
                    nc.scalar.activation(g[:], p_u[:], AF.Gelu,
                                         bias=dwb2[:, li:li + 1], scale=1.0)
                    p_pw = psB.tile([P, C], F32, tag="b")
                    mm(p_pw, pwt(li), g[:], start=True, stop=True)
                    last = (li == 3)
                    hw_ = C if last else HALO + C
                    h_nx = hpools[li + 1].tile([P, hw_], F32, tag=f"h{li+1}")
                    off = 0 if last else HALO
                    nc.vector.scalar_tensor_tensor(
                        h_nx[:, off:], p_pw[:], pwb2[:, li:li + 1],
                        h_in[:, HALO:], OP.add, OP.add)
                    if not last:
                        if j == 0:
                            nc.vector.memset(h_nx[:, 0:HALO], 0.0)
                        else:
                            nc.vector.tensor_copy(h_nx[:, 0:HALO],
                                                  prev_h[li + 1][:, C:C + HALO])
                    prev_h[li] = h_in
                    h_in = h_nx
                h4 = h_in  # [P, C]

                # ---- layernorm ----
                p_t = psA.tile([P, C], F32, tag="a")
                for b in range(NB):
                    mm(p_t, h4[:, b * 128:(b + 1) * 128], ident[:],
                       start=True, stop=True, col0=b * 128)
                p_tm = psS.tile([P, 2 * NB], F32, tag="s")
                for b in range(NB):
                    nc.tensor.matmul(p_tm[:, 2 * b:2 * b + 2],
                                     M2[:, b * 128:(b + 1) * 128], i2[:],
                                     start=True, stop=True)

                s1 = spool.tile([P, 2 * NB], F32, tag="s1")
                nc.vector.tensor_reduce(
                    s1[:].rearrange("p (b q) -> p b q", q=2),
                    p_t[:].rearrange("p (b q c) -> p b q c", q=2, c=DIM),
                    AX.X, OP.add)
                xsq = gpool.tile([P, C], F32, tag="xsq")
                nc.scalar.activation(xsq[:], p_t[:], AF.Square)
                s2 = spool.tile([P, 2 * NB], F32, tag="s2")
                nc.vector.tensor_reduce(
                    s2[:].rearrange("p (b q) -> p b q", q=2),
                    xsq[:].rearrange("p (b q c) -> p b q c", q=2, c=DIM),
                    AX.X, OP.add)

                mu = spool.tile([P, 2 * NB], F32, tag="mu")
                nc.vector.tensor_scalar(mu[:], s1[:], 1.0 / DIM, None, OP.mult)
                var = spool.tile([P, 2 * NB], F32, tag="var")
                nc.vector.tensor_mul(var[:], mu[:], mu[:])
                nc.vector.scalar_tensor_tensor(
                    var[:], s2[:], 1.0 / DIM, var[:], OP.mult, OP.subtract)
                stdv = spool.tile([P, 2 * NB], F32, tag="stdv")
                nc.scalar.activation(stdv[:], var[:], AF.Sqrt, bias=epsc[:])
                rstd = spool.tile([P, 2 * NB], F32, tag="rstd")
                nc.vector.reciprocal(rstd[:], stdv[:])
                # AC tile [P, 4*NB], col groups: [A_q0(b), A_q1(b), C0A_q0(b), C0A_q1(b)]
                ac = spool.tile([P, 4 * NB], F32, tag="ac")
                acA = ac[:, 0:2 * NB].rearrange("p (q b) -> p b q", q=2)
                acC = ac[:, 2 * NB:4 * NB].rearrange("p (q b) -> p b q", q=2)
                nc.vector.tensor_mul(
                    acA, rstd[:].rearrange("p (b q) -> p b q", q=2),
                    p_tm[:].rearrange("p (b q) -> p b q", q=2))
                nc.vector.scalar_tensor_tensor(
                    acC, mu[:].rearrange("p (b q) -> p b q", q=2), -1.0, acA,
                    OP.mult, OP.mult)
                # transpose AC -> rows, evict, re-partition to [2, C] tiles
                pac = psS.tile([4 * NB, P], F32, tag="s")
                nc.tensor.matmul(pac[:], ac[:], ident[:], start=True, stop=True)
                acr = spool.tile([4 * NB, P], F32, tag="acr")
                nc.scalar.copy(acr[:], pac[:])
                acrA = spool.tile([2, C], F32, tag="acrA")
                acrC = spool.tile([2, C], F32, tag="acrC")
                for gq in range(2):
                    nc.sync.dma_start(acrA[gq:gq + 1, :],
                                      acr[gq * NB:(gq + 1) * NB, :])
                    nc.sync.dma_start(acrC[gq:gq + 1, :],
                                      acr[(2 + gq) * NB:(3 + gq) * NB, :])

                # broadcast matmuls (full-width)
                p_h2 = psA.tile([P, C], F32, tag="a")    # A_bc
                p_m2 = psB.tile([P, C], F32, tag="b")    # C0A*g + m*b
                mm(p_h2, sel2[:], acrA[:], start=True, stop=True)
                mm(p_m2, gsel[:], acrC[:], start=True, stop=False)
                mm(p_m2, bsel[:], M2[:], start=False, stop=True)

                o1 = opool.tile([P, C], F32, tag="o1")
                nc.vector.tensor_scalar(o1[:], h4[:], g2col[:], None, OP.mult)
                o2 = opool.tile([P, C], F32, tag="o2")
                nc.vector.tensor_mul(o2[:], o1[:], p_h2[:])
                o3 = opool.tile([P, C], F32, tag="o3")
                nc.vector.tensor_add(o3[:], o2[:], p_m2[:])

                p_o = psO.tile([P, C], F32, tag="o")
                for b in range(NB):
                    mm(p_o, o3[:, b * 128:(b + 1) * 128], ident[:],
                       start=True, stop=True, col0=b * 128)
                outt = outpool.tile([P, C], F32, tag="outt")
                nc.scalar.copy(outt[:], p_o[:])

                for q, brow in ((0, b0), (1, b1)):
                    nc.sync.dma_start(
                        d_out[brow, t0:t0 + C, :].rearrange("(b t) c -> t b c", t=128),
                        outt[:].rearrange("p (b q c) -> p b q c", q=2, c=DIM)[:, :, q, :])

    nc.compile()
    return nc


def host_prep(inputs: dict, n_cores: int = N_CORES, T: int = T_FULL,
              C: int = C_CHUNK):
    f32 = np.float32
    ids = np.asarray(inputs["unit_ids"]).astype(np.int64)
    anchor = np.ascontiguousarray(np.asarray(inputs["log_anchor"], dtype=f32))
    rate = np.asarray(inputs["source_rate"], dtype=f32)
    sil = np.clip(np.asarray(inputs["silence_mask"], dtype=f32), 0.0, 1.0)
    mask = np.clip(np.asarray(inputs["unit_mask"], dtype=f32), 0.0, 1.0)
    emb = np.asarray(inputs["emb"], dtype=f32)
    in_w = np.asarray(inputs["in_w"], dtype=f32)
    in_b = np.asarray(inputs["in_b"], dtype=f32)
    dw_w = np.asarray(inputs["dw_w"], dtype=f32)
    dw_b = np.asarray(inputs["dw_b"], dtype=f32)
    pw_w = np.asarray(inputs["pw_w"], dtype=f32)
    pw_b = np.asarray(inputs["pw_b"], dtype=f32)
    ln_g = np.asarray(inputs["ln_g"], dtype=f32)
    ln_b = np.asarray(inputs["ln_b"], dtype=f32)

    B = ids.shape[0]
    assert B % n_cores == 0
    bpc = B // n_cores
    NCH = T // C
    NS = C // 16

    E2 = emb @ in_w[:, :DIM].T
    etab = np.ascontiguousarray(np.concatenate([E2.T, E2.T], axis=0)).astype(f32)
    cidx = np.arange(P) % DIM
    w6 = np.zeros((6, P), f32)
    wa, wc, ws = in_w[:, DIM], in_w[:, DIM + 1], in_w[:, DIM + 2]
    w6[0, :DIM], w6[1, :DIM], w6[2, :DIM] = wa, wc, ws
    w6[3, DIM:], w6[4, DIM:], w6[5, DIM:] = wa, wc, ws
    sel2 = np.zeros((2, P), f32)
    sel2[0, :DIM] = 1.0
    sel2[1, DIM:] = 1.0
    inb2 = in_b[cidx].reshape(P, 1).astype(f32)
    dtaps = np.zeros((12, P, P), f32)
    for i in range(4):
        for k in range(3):
            np.fill_diagonal(dtaps[i * 3 + k], dw_w[i, cidx, 0, k])
    dtaps = np.ascontiguousarray(dtaps.transpose(1, 0, 2).reshape(P, 12 * P))
    pw2 = np.zeros((4, P, P), f32)
    for i in range(4):
        pw2[i, :DIM, :DIM] = pw_w[i].T
        pw2[i, DIM:, DIM:] = pw_w[i].T
    pw2 = np.ascontiguousarray(pw2.transpose(1, 0, 2).reshape(P, 4 * P))
    dwb2 = np.ascontiguousarray(dw_b[:, cidx].T).astype(f32)
    pwb2 = np.ascontiguousarray(pw_b[:, cidx].T).astype(f32)
    ident = np.eye(P, dtype=f32)
    i2 = np.eye(2, dtype=f32)
    gsel = (sel2 * ln_g[cidx][None, :]).astype(f32)
    bsel = (sel2 * ln_b[cidx][None, :]).astype(f32)
    g2col = ln_g[cidx].reshape(P, 1).astype(f32)

    consts = dict(etab=etab, w6=w6, sel2=sel2, inb2=inb2, dtaps=dtaps,
                  pw2=pw2, dwb2=dwb2, pwb2=pwb2, ident=ident, i2=i2,
                  gsel=gsel, bsel=bsel, g2col=g2col)

    ids16 = ids.astype(np.int16)
    in_maps = []
    for core in range(n_cores):
        rows = slice(core * bpc, (core + 1) * bpc)
        # wrapped idx layout: [pair, chunk, 128, NS]; 16-part group g uses
        # batch row (pair*2 + g//4); within group, idx j=s*16+lane at [lane, s]
        idxw = np.zeros((2, NCH, P, NS), np.int16)
        seg = ids16[rows].reshape(bpc, NCH, NS, 16)       # [4, NCH, s, lane]
        segT = seg.transpose(0, 1, 3, 2)                  # [4, NCH, lane, s]
        for pair in range(2):
            for grp in range(8):
                brow = 2 * pair + (0 if grp < 4 else 1)
                idxw[pair, :, grp * 16:(grp + 1) * 16, :] = segT[brow]
        cent = (anchor[rows] - rate[rows]) * mask[rows]
        silm = sil[rows] * mask[rows]
        in_maps.append(dict(anchor=anchor[rows], rate=cent, sil=silm,
                            mask=mask[rows], idxw=idxw, **consts))
    return in_maps


_PROGRAM_CACHE = {}


def _get_program(T, C):
    key = (T, C)
    if key not in _PROGRAM_CACHE:
        _PROGRAM_CACHE[key] = build_program(T=T, C=C)
    return _PROGRAM_CACHE[key]


def kernel(**inputs) -> np.ndarray:
    from concourse.bass_utils import run_bass_kernel_spmd
    T, C = T_FULL, C_CHUNK
    nc = _get_program(T, C)
    in_maps = host_prep(inputs, n_cores=N_CORES, T=T, C=C)
    res = run_bass_kernel_spmd(nc, in_maps, list(range(N_CORES)))
    outs = [np.asarray(res.results[i]["out"]) for i in range(N_CORES)]
    return np.concatenate(outs, axis=0)  # [32, 16384, 64] f32
